# revision 39
# baseline (speedup 1.0000x reference)
"""Trainium2 Bass kernel for nn_ANNSimple (spline-fit + eval + tiny MLP + MSE).

Math: the reference's cubic-spline fit (not-a-knot) and evaluation at the two
fixed scalars i, j are linear maps of each row of x, so there are fixed
66-vectors g_i, g_j with r_i = x @ g_i and r_j = x @ g_j, and

    ndis    = (r_i - r_j) / (r_i + r_j)                 (per row)
    h1      = W1 @ ndis + b1 ; LeakyReLU(0.01)
    soc_hat = W2 @ h1 + b2
    loss    = sum((soc_hat - soc)^2)
            = N*c^2 - 2*c*sum(soc) + sum(soc^2),  c = soc_hat[0]

g_i / g_j decay exponentially away from the interval containing their eval
point (tridiagonal not-a-knot system), so each is supported on ~24 columns.
Only those columns of x are shipped to the device, and r_i / r_j are exact
f32 dot products over their own windows (full f32 precision — the denominator
r_i + r_j crosses zero for rare rows, which rules out bf16/TensorE-fp32 paths).

Distribution: pure data parallel over rows across 8 NeuronCores, slab layout
(SBUF partition p owns rows [p*512, (p+1)*512) of the core's shard).  Per core:
  - VectorE: one multiply pass x*g over the windowed columns, two windowed
    reduces -> r_i, r_j; num/den/reciprocal/ndis; W1 @ ndis partials and
    sum(soc), sum(soc^2) via multiply+reduce.
  - TensorE: ones-matmul partition reduction -> 12 partials.
  - Final reduction of the 12 partial sums (10x W1-partials, sum(soc),
    sum(soc^2)): by default they are summed across cores on the host during
    the gather/unshard step (96 bytes/core; the tiny MLP tail is ~30 flops).
    BASS_USE_CC=1 instead runs the 12-float AllReduce + scalar tail on
    device -- it is correct but adds ~20-40us: the first collective in a
    NEFF pays a global cross-core rendezvous barrier that does not fully
    overlap (a warm-up collective at kernel start absorbs part of it).
"""
import os
import sys

if "/opt/trn_rl_repo" not in sys.path:
    sys.path.insert(0, "/opt/trn_rl_repo")

import numpy as np

from concourse import bacc, mybir, tile
from concourse.bass_utils import run_bass_kernel_spmd

NTOT = 524288
K = 66
NCORES = 8
R = NTOT // NCORES  # 65536 rows per core
P = 128
SLAB = R // P  # 512 rows per partition
FTS = [128, 128, 128, 128]  # rows/partition per x tile (sum = SLAB)
NT = len(FTS)
FOFF = [sum(FTS[:t]) for t in range(NT)]
assert sum(FTS) == SLAB

f32 = mybir.dt.float32

# ---------------------------------------------------------------- host math
_xk = np.linspace(0.0, 1.0, K).astype(np.float64)
_dx = np.diff(_xk)


def _notaknot_matrix():
    A = np.zeros((K, K), np.float64)
    i = np.arange(1, K - 1)
    A[i, i - 1] = _dx[1:]
    A[i, i] = 2.0 * (_dx[1:] + _dx[:-1])
    A[i, i + 1] = _dx[:-1]
    A[0, 0] = _dx[1]
    A[0, 1] = _xk[2] - _xk[0]
    A[-1, -1] = _dx[-2]
    A[-1, -2] = _xk[-1] - _xk[-3]
    return A


def _build_g(t: float) -> np.ndarray:
    """g (66,) float64 with: spline_eval(row y, t) == y @ g."""
    invA = np.linalg.inv(_notaknot_matrix())
    S = np.zeros((K, K - 1))
    for m in range(K - 1):
        S[m, m] = -1.0 / _dx[m]
        S[m + 1, m] = 1.0 / _dx[m]
    Cs = np.zeros((K - 1, K))
    d02 = _xk[2] - _xk[0]
    dl = _xk[-1] - _xk[-3]
    Cs[0, 0] = (_dx[0] + 2.0 * d02) * _dx[1] / d02
    Cs[1, 0] = _dx[0] * _dx[0] / d02
    for m in range(K - 2):
        Cs[m, m + 1] = 3.0 * _dx[m + 1]
        Cs[m + 1, m + 1] = 3.0 * _dx[m]
    Cs[K - 3, K - 1] = _dx[-1] * _dx[-1] / dl
    Cs[K - 2, K - 1] = (2.0 * dl + _dx[-1]) * _dx[-2] / dl
    B = S @ Cs
    k = int(np.clip(np.searchsorted(_xk, t, side="right") - 1, 0, K - 2))
    h = _dx[k]
    u = t - _xk[k]
    v_k = B @ invA[k, :]
    v_k1 = B @ invA[k + 1, :]
    w_sk = u**3 / h**2 - 2 * u**2 / h + u
    w_sk1 = u**3 / h**2 - u**2 / h
    w_mk = -2 * u**3 / h**2 + 3 * u**2 / h
    g = w_sk * v_k + w_sk1 * v_k1 + w_mk * S[:, k]
    g[k] += 1.0
    return g


def _window(g: np.ndarray, tau: float = 1e-8) -> tuple:
    """Smallest even-width contiguous [lo, hi) covering {|g| > tau}."""
    idx = np.where(np.abs(g) > tau)[0]
    lo, hi = int(idx.min()), int(idx.max()) + 1
    if (hi - lo) % 2:
        if hi < len(g):
            hi += 1
        else:
            lo -= 1
    return lo, hi


# ---------------------------------------------------------------- program
_PROGRAM_CACHE: dict = {}


def _build_program(C: int, Ci: int, use_cc: bool, variant: int = 0):
    """C = total windowed columns, Ci = columns in the g_i block (rest g_j)."""
    fts = [64, 64, 128, 128, 128] if variant == 8 else FTS
    nt = len(fts)
    foff = [sum(fts[:t]) for t in range(nt)]
    nc = bacc.Bacc(
        "TRN2", target_bir_lowering=False, debug=False, num_devices=NCORES
    )

    xc_ext = nc.declare_dram_parameter("xc", [R, C], f32, isOutput=False)
    if variant == 10:
        x0i_ext = nc.declare_dram_parameter(
            "x0i", [P, FTS[0], Ci], f32, isOutput=False
        )
        x0j_ext = nc.declare_dram_parameter(
            "x0j", [P, FTS[0], C - Ci], f32, isOutput=False
        )
    g2_ext = nc.declare_dram_parameter("g2", [P, C], f32, isOutput=False)
    w1_ext = nc.declare_dram_parameter("w1", [P, 10 * SLAB], f32, isOutput=False)
    soc_ext = nc.declare_dram_parameter("soc", [P, SLAB], f32, isOutput=False)
    b1w2_ext = nc.declare_dram_parameter("b1w2", [1, 20], f32, isOutput=False)
    b2_ext = nc.declare_dram_parameter("b2", [1, 1], f32, isOutput=False)
    # out: [0:R) ndis (natural row order); [R:R+12) partials; [R+12:R+14) finals
    out_ext = nc.declare_dram_parameter("out", [R + 14], f32, isOutput=True)

    if use_cc:
        cc_in = nc.dram_tensor("cc_in", [12], f32)
        cc_out = nc.dram_tensor("cc_out", [12], f32, addr_space="Shared")
        ccw_in = nc.dram_tensor("ccw_in", [4], f32)
        ccw_out = nc.dram_tensor("ccw_out", [4], f32, addr_space="Shared")

    add = mybir.AluOpType.add
    mult = mybir.AluOpType.mult
    X = mybir.AxisListType.X

    with tile.TileContext(nc) as tc:
        with (
            tc.tile_pool(name="const", bufs=1) as cpool,
            tc.tile_pool(name="xc", bufs=2 if variant == 10 else 3) as xpool,
            tc.tile_pool(name="prod", bufs=2) as ppool,
            tc.tile_pool(name="big", bufs=1) as big,
            tc.tile_pool(name="work", bufs=2) as work,
            tc.tile_pool(name="psred", bufs=1, space="PSUM") as psred,
        ):
            ones_t = cpool.tile([P, 1], f32)
            nc.gpsimd.memset(ones_t[:], 1.0)

            if use_cc:
                # warm up the collective path: the first collective pays a
                # global cross-core rendezvous barrier; run it on 16 bytes
                # now so it overlaps the main compute.
                warm = cpool.tile([1, 4], f32)
                nc.gpsimd.memset(warm[:], 0.0)
                nc.sync.dma_start(out=ccw_in[:], in_=warm[:].rearrange("o x -> (o x)"))
                nc.gpsimd.collective_compute(
                    "AllReduce",
                    mybir.AluOpType.add,
                    replica_groups=[list(range(NCORES))],
                    ins=[ccw_in[:].opt()],
                    outs=[ccw_out[:].opt()],
                )

            xsrc = xc_ext[:].rearrange("(p f) c -> p f c", p=P)
            x_tiles = {}

            def load_x(t):
                FT = fts[t]
                fs = slice(foff[t], foff[t] + FT)
                xt = xpool.tile([P, FT, C], f32, tag="x", name=f"x{t}")
                nc.sync.dma_start(out=xt[:], in_=xsrc[:, fs, :])
                return xt

            g2_t = cpool.tile([P, C], f32, name="g2t")
            nc.sync.dma_start(out=g2_t[:], in_=g2_ext[:])
            b1w2_t = cpool.tile([1, 20], f32, name="b1w2t")
            nc.sync.dma_start(out=b1w2_t[:], in_=b1w2_ext[:])
            b2_t = cpool.tile([1, 1], f32, name="b2t")
            nc.sync.dma_start(out=b2_t[:], in_=b2_ext[:])
            if variant == 10:
                x0i_t = xpool.tile([P, fts[0], Ci], f32, name="x0i")
                nc.sync.dma_start(out=x0i_t[:], in_=x0i_ext[:])
                x0j_t = xpool.tile([P, fts[0], C - Ci], f32, name="x0j")
                nc.sync.dma_start(out=x0j_t[:], in_=x0j_ext[:])
                x_tiles[0] = None
                for t in range(1, 3):
                    x_tiles[t] = load_x(t)
            else:
                for t in range(3):
                    x_tiles[t] = load_x(t)

            w1_t = big.tile([P, 10 * SLAB], f32)
            nc.sync.dma_start(out=w1_t[:], in_=w1_ext[:])
            soc_t = big.tile([P, SLAB], f32)
            nc.sync.dma_start(out=soc_t[:], in_=soc_ext[:])

            ri_t = big.tile([P, SLAB], f32)
            rj_t = big.tile([P, SLAB], f32)
            ndis_t = big.tile([P, SLAB], f32)

            NHP = 2 if variant in (5, 6, 7, 8, 9, 10, 11) else nt
            hp = [cpool.tile([P, 12], f32, name=f"hp{t}") for t in range(NHP)]
            for t in range(1, NHP):
                nc.gpsimd.memset(hp[t][:, 10:12], 0.0)

            # soc stats (independent of the spline path)
            nc.vector.tensor_reduce(hp[0][:, 10:11], soc_t[:], X, add)
            s2scr = work.tile([P, SLAB], f32, tag="s2scr")
            nc.gpsimd.tensor_tensor(s2scr[:], soc_t[:], soc_t[:], mult)
            nc.vector.tensor_reduce(hp[0][:, 11:12], s2scr[:], X, add)

            for t in range(nt):
                FT = fts[t]
                fs = slice(foff[t], foff[t] + FT)
                if t in x_tiles:
                    x_t = x_tiles.pop(t)
                else:
                    x_t = load_x(t)
                if x_t is None:
                    x_t = x0i_t  # variant 10, t=0: real sources picked below
                g_bi = g2_t[:, 0:Ci].unsqueeze(1).broadcast_to((P, FT, Ci))
                g_bj = g2_t[:, Ci:C].unsqueeze(1).broadcast_to((P, FT, C - Ci))
                if variant == 5:
                    # GP owns the product pass; DVE owns the reduce chain.
                    # Last tile: DVE takes one window back for balance.
                    if t < nt - 1:
                        prod = ppool.tile([P, FT, C], f32, tag="prod")
                        g_b = g2_t[:].unsqueeze(1).broadcast_to((P, FT, C))
                        nc.gpsimd.tensor_tensor(prod[:], x_t[:], g_b, mult)
                        ri_src = prod[:, :, 0:Ci]
                        rj_src = prod[:, :, Ci:C]
                    else:
                        prodi = ppool.tile([P, FT, Ci], f32, tag="prodi")
                        prodj = ppool.tile([P, FT, C - Ci], f32, tag="prodj")
                        nc.vector.tensor_tensor(prodi[:], x_t[:, :, 0:Ci], g_bi, mult)
                        nc.gpsimd.tensor_tensor(prodj[:], x_t[:, :, Ci:C], g_bj, mult)
                        ri_src = prodi[:]
                        rj_src = prodj[:]
                elif variant == 9:
                    # DVE multiplies only SPL columns; GP takes the rest.
                    SPL = 10
                    prodi = ppool.tile([P, FT, SPL], f32, tag="prodi")
                    prodb = ppool.tile([P, FT, C - SPL], f32, tag="prodb")
                    g_ba = g2_t[:, 0:SPL].unsqueeze(1).broadcast_to((P, FT, SPL))
                    g_bb = g2_t[:, SPL:C].unsqueeze(1).broadcast_to(
                        (P, FT, C - SPL)
                    )
                    nc.vector.tensor_tensor(prodi[:], x_t[:, :, 0:SPL], g_ba, mult)
                    nc.gpsimd.tensor_tensor(prodb[:], x_t[:, :, SPL:C], g_bb, mult)
                    ria = work.tile([P, FT], f32, tag="ria")
                    rib = work.tile([P, FT], f32, tag="rib")
                    nc.vector.tensor_reduce(ria[:], prodi[:], X, add)
                    nc.vector.tensor_reduce(
                        rib[:], prodb[:, :, 0 : Ci - SPL], X, add
                    )
                    nc.vector.tensor_tensor(ri_t[:, fs], ria[:], rib[:], add)
                    nc.vector.tensor_reduce(
                        rj_t[:, fs], prodb[:, :, Ci - SPL : C - SPL], X, add
                    )
                    ri_src = None
                else:
                    prodi = ppool.tile([P, FT, Ci], f32, tag="prodi")
                    prodj = ppool.tile([P, FT, C - Ci], f32, tag="prodj")
                    if variant == 10 and t == 0:
                        xi_src, xj_src = x0i_t[:], x0j_t[:]
                    else:
                        xi_src, xj_src = x_t[:, :, 0:Ci], x_t[:, :, Ci:C]
                    nc.vector.tensor_tensor(prodi[:], xi_src, g_bi, mult)
                    nc.gpsimd.tensor_tensor(prodj[:], xj_src, g_bj, mult)
                    ri_src = prodi[:]
                    rj_src = prodj[:]
                if ri_src is not None:
                    nc.vector.tensor_reduce(ri_t[:, fs], ri_src, X, add)
                    nc.vector.tensor_reduce(rj_t[:, fs], rj_src, X, add)
                num = work.tile([P, FT], f32, tag="num")
                den = work.tile([P, FT], f32, tag="den")
                ndeng = nc.vector if variant in (5, 7) else nc.gpsimd
                ndeng.tensor_tensor(
                    num[:], ri_t[:, fs], rj_t[:, fs], mybir.AluOpType.subtract
                )
                ndeng.tensor_tensor(den[:], ri_t[:, fs], rj_t[:, fs], add)
                if variant == 11:
                    nc.gpsimd.tensor_tensor(
                        ndis_t[:, fs], num[:], den[:], mybir.AluOpType.divide
                    )
                else:
                    rinv = work.tile([P, FT], f32, tag="rinv")
                    nc.vector.reciprocal(rinv[:], den[:])
                    nc.vector.tensor_tensor(ndis_t[:, fs], num[:], rinv[:], mult)
                # W1 @ ndis partials
                if variant in (5, 6, 7, 8, 9, 10, 11):
                    end = foff[t] + FT
                    if end in (SLAB // 2, SLAB):
                        h = 0 if end <= SLAB // 2 else 1
                        hs = slice(h * (SLAB // 2), end)
                        HW = hs.stop - hs.start
                        for m in range(10):
                            scr = work.tile([P, HW], f32, tag="scr2")
                            nc.vector.tensor_tensor(
                                scr[:],
                                w1_t[:, m * SLAB + hs.start : m * SLAB + hs.stop],
                                ndis_t[:, hs],
                                mult,
                            )
                            nc.vector.tensor_reduce(
                                hp[h][:, m : m + 1], scr[:], X, add
                            )
                else:
                    for m in range(10):
                        scr = work.tile([P, FT], f32, tag="scr2")
                        nc.gpsimd.tensor_tensor(
                            scr[:],
                            w1_t[:, m * SLAB + foff[t] : m * SLAB + foff[t] + FT],
                            ndis_t[:, fs],
                            mult,
                        )
                        nc.vector.tensor_reduce(hp[t][:, m : m + 1], scr[:], X, add)

            nc.sync.dma_start(
                out=out_ext[0:R].rearrange("(p f) -> p f", p=P), in_=ndis_t[:]
            )

            # combine per-tile partials, reduce across partitions via TensorE
            acc_hp = hp[0]
            for t in range(1, NHP):
                nxt = cpool.tile([P, 12], f32, name=f"hpacc{t}")
                nc.vector.tensor_tensor(nxt[:], acc_hp[:], hp[t][:], add)
                acc_hp = nxt
            hptot = acc_hp
            pred = psred.tile([12, 1], f32)
            nc.tensor.matmul(pred[:], hptot[:], ones_t[:], start=True, stop=True)
            predsb = cpool.tile([12, 1], f32)
            nc.vector.tensor_copy(predsb[:], pred[:])
            nc.sync.dma_start(
                out=out_ext[R : R + 12].rearrange("(p o) -> p o", o=1),
                in_=predsb[:],
            )

            if use_cc:
                nc.sync.dma_start(
                    out=cc_in[:], in_=predsb[:].rearrange("p o -> (p o)")
                )
                nc.gpsimd.collective_compute(
                    "AllReduce",
                    mybir.AluOpType.add,
                    replica_groups=[list(range(NCORES))],
                    ins=[cc_in[:].opt()],
                    outs=[cc_out[:].opt()],
                )
                redt = cpool.tile([1, 12], f32)
                nc.sync.dma_start(out=redt[:], in_=cc_out[:].unsqueeze(0))
                # tail: h1 = partials + b1 ; leaky ; soc_hat = W2 @ h1 + b2
                h1 = cpool.tile([1, 10], f32)
                nc.vector.tensor_tensor(h1[:], redt[:, 0:10], b1w2_t[:, 0:10], add)
                lk = cpool.tile([1, 10], f32)
                nc.vector.tensor_scalar_mul(lk[:], h1[:], 0.01)
                h1a = cpool.tile([1, 10], f32)
                nc.vector.tensor_tensor(h1a[:], h1[:], lk[:], mybir.AluOpType.max)
                pr = cpool.tile([1, 10], f32)
                nc.vector.tensor_tensor(pr[:], h1a[:], b1w2_t[:, 10:20], mult)
                fin = cpool.tile([1, 2], f32)
                hs = cpool.tile([1, 1], f32)
                nc.vector.tensor_reduce(hs[:], pr[:], X, add)
                nc.vector.tensor_tensor(fin[:, 0:1], hs[:], b2_t[:], add)
                # loss = c*(N*c - 2*S1) + S2
                t1 = cpool.tile([1, 1], f32)
                nc.vector.tensor_scalar_mul(t1[:], fin[:, 0:1], float(NTOT))
                t2 = cpool.tile([1, 1], f32)
                nc.vector.tensor_scalar_mul(t2[:], redt[:, 10:11], 2.0)
                t3 = cpool.tile([1, 1], f32)
                nc.vector.tensor_tensor(
                    t3[:], t1[:], t2[:], mybir.AluOpType.subtract
                )
                t4 = cpool.tile([1, 1], f32)
                nc.vector.tensor_tensor(t4[:], t3[:], fin[:, 0:1], mult)
                nc.vector.tensor_tensor(fin[:, 1:2], t4[:], redt[:, 11:12], add)
                nc.sync.dma_start(
                    out=out_ext[R + 12 : R + 14].unsqueeze(0), in_=fin[:]
                )

    nc.compile()
    return nc


# ---------------------------------------------------------------- entry
def kernel(x, soc, i, j, W1, b1, W2, b2):
    x = np.asarray(x, dtype=np.float32)
    soc = np.ascontiguousarray(np.asarray(soc, dtype=np.float32))
    W1 = np.ascontiguousarray(np.asarray(W1, dtype=np.float32))
    b1 = np.asarray(b1, dtype=np.float32)
    W2 = np.asarray(W2, dtype=np.float32)
    b2 = np.asarray(b2, dtype=np.float32)
    fi, fj = float(np.asarray(i)), float(np.asarray(j))

    gi = _build_g(fi)
    gj = _build_g(fj)
    ilo, ihi = _window(gi)
    jlo, jhi = _window(gj)
    Ci, Cj = ihi - ilo, jhi - jlo
    C = Ci + Cj
    giw = gi[ilo:ihi].astype(np.float32)
    gjw = gj[jlo:jhi].astype(np.float32)

    use_cc = bool(int(os.environ.get("BASS_USE_CC", "0")))
    variant = int(os.environ.get("BASS_KVARIANT", "6"))
    key = (C, Ci, use_cc, variant)
    nc = _PROGRAM_CACHE.get(key)
    if nc is None:
        nc = _build_program(C, Ci, use_cc, variant)
        _PROGRAM_CACHE[key] = nc

    g2_in = np.ascontiguousarray(
        np.broadcast_to(np.concatenate([giw, gjw])[None, :], (P, C))
    )
    b1w2_in = np.concatenate([b1, W2.reshape(-1)]).reshape(1, 20)
    b2_in = b2.reshape(1, 1)

    xc = np.empty((NTOT, C), np.float32)
    xc[:, :Ci] = x[:, ilo:ihi]
    xc[:, Ci:] = x[:, jlo:jhi]

    in_maps = []
    for c in range(NCORES):
        rs = slice(c * R, (c + 1) * R)
        extra = {}
        if variant == 10:
            xv = xc[rs].reshape(P, SLAB, C)
            extra["x0i"] = np.ascontiguousarray(xv[:, : FTS[0], :Ci])
            extra["x0j"] = np.ascontiguousarray(xv[:, : FTS[0], Ci:])
        in_maps.append(
            {
                **extra,
                "xc": xc[rs],
                "g2": g2_in,
                "w1": np.ascontiguousarray(
                    W1[:, rs].reshape(10, P, SLAB).transpose(1, 0, 2)
                ).reshape(P, 10 * SLAB),
                "soc": soc[rs].reshape(P, SLAB),
                "b1w2": b1w2_in,
                "b2": b2_in,
            }
        )

    res = run_bass_kernel_spmd(
        nc,
        in_maps,
        core_ids=list(range(NCORES)),
        trace=bool(int(os.environ.get("BASS_TRACE", "0"))),
    )
    if res.exec_time_ns is not None:
        print(f"HW exec time: {res.exec_time_ns} ns")

    ndis = np.empty(NTOT, dtype=np.float32)
    for c in range(NCORES):
        ndis[c * R : (c + 1) * R] = res.results[c]["out"][:R]

    if use_cc:
        soc_hat = np.array([res.results[0]["out"][R + 12]], dtype=np.float32)
        loss = np.float32(res.results[0]["out"][R + 13])
    else:
        partials = np.sum(
            [res.results[c]["out"][R : R + 12] for c in range(NCORES)], axis=0
        ).astype(np.float32)
        h1 = partials[:10] + b1
        h1 = np.where(h1 > 0, h1, np.float32(0.01) * h1).astype(np.float32)
        c0 = np.float32(W2.reshape(-1) @ h1 + b2[0])
        soc_hat = np.array([c0], dtype=np.float32)
        loss = np.float32(
            np.float32(NTOT) * c0 * c0
            - np.float32(2.0) * c0 * partials[10]
            + partials[11]
        )

    return soc_hat, ndis, np.asarray(loss, dtype=np.float32)


# revision 40
# speedup vs baseline: 1.0722x; 1.0722x over previous
"""Trainium2 Bass kernel for nn_ANNSimple (spline-fit + eval + tiny MLP + MSE).

Math: the reference's cubic-spline fit (not-a-knot) and evaluation at the two
fixed scalars i, j are linear maps of each row of x, so there are fixed
66-vectors g_i, g_j with r_i = x @ g_i and r_j = x @ g_j, and

    ndis    = (r_i - r_j) / (r_i + r_j)                 (per row)
    h1      = W1 @ ndis + b1 ; LeakyReLU(0.01)
    soc_hat = W2 @ h1 + b2
    loss    = sum((soc_hat - soc)^2)
            = N*c^2 - 2*c*sum(soc) + sum(soc^2),  c = soc_hat[0]

g_i / g_j decay exponentially away from the interval containing their eval
point (tridiagonal not-a-knot system), so each is supported on ~24 columns.
Only those columns of x are shipped to the device, and r_i / r_j are exact
f32 dot products over their own windows (full f32 precision — the denominator
r_i + r_j crosses zero for rare rows, which rules out bf16/TensorE-fp32 paths).

Distribution: pure data parallel over rows across 8 NeuronCores, slab layout
(SBUF partition p owns rows [p*512, (p+1)*512) of the core's shard).  Per core:
  - VectorE: one multiply pass x*g over the windowed columns, two windowed
    reduces -> r_i, r_j; num/den/reciprocal/ndis; W1 @ ndis partials and
    sum(soc), sum(soc^2) via multiply+reduce.
  - TensorE: ones-matmul partition reduction -> 12 partials.
  - Final reduction of the 12 partial sums (10x W1-partials, sum(soc),
    sum(soc^2)): by default they are summed across cores on the host during
    the gather/unshard step (96 bytes/core; the tiny MLP tail is ~30 flops).
    BASS_USE_CC=1 instead runs the 12-float AllReduce + scalar tail on
    device -- it is correct but adds ~20-40us: the first collective in a
    NEFF pays a global cross-core rendezvous barrier that does not fully
    overlap (a warm-up collective at kernel start absorbs part of it).
"""
import os
import sys

if "/opt/trn_rl_repo" not in sys.path:
    sys.path.insert(0, "/opt/trn_rl_repo")

import numpy as np

from concourse import bacc, mybir, tile
from concourse.bass_utils import run_bass_kernel_spmd

NTOT = 524288
K = 66
NCORES = 8
R = NTOT // NCORES  # 65536 rows per core
P = 128
SLAB = R // P  # 512 rows per partition
FTS = [128, 128, 128, 128]  # rows/partition per x tile (sum = SLAB)
NT = len(FTS)
FOFF = [sum(FTS[:t]) for t in range(NT)]
assert sum(FTS) == SLAB

f32 = mybir.dt.float32

# ---------------------------------------------------------------- host math
_xk = np.linspace(0.0, 1.0, K).astype(np.float64)
_dx = np.diff(_xk)


def _notaknot_matrix():
    A = np.zeros((K, K), np.float64)
    i = np.arange(1, K - 1)
    A[i, i - 1] = _dx[1:]
    A[i, i] = 2.0 * (_dx[1:] + _dx[:-1])
    A[i, i + 1] = _dx[:-1]
    A[0, 0] = _dx[1]
    A[0, 1] = _xk[2] - _xk[0]
    A[-1, -1] = _dx[-2]
    A[-1, -2] = _xk[-1] - _xk[-3]
    return A


def _build_g(t: float) -> np.ndarray:
    """g (66,) float64 with: spline_eval(row y, t) == y @ g."""
    invA = np.linalg.inv(_notaknot_matrix())
    S = np.zeros((K, K - 1))
    for m in range(K - 1):
        S[m, m] = -1.0 / _dx[m]
        S[m + 1, m] = 1.0 / _dx[m]
    Cs = np.zeros((K - 1, K))
    d02 = _xk[2] - _xk[0]
    dl = _xk[-1] - _xk[-3]
    Cs[0, 0] = (_dx[0] + 2.0 * d02) * _dx[1] / d02
    Cs[1, 0] = _dx[0] * _dx[0] / d02
    for m in range(K - 2):
        Cs[m, m + 1] = 3.0 * _dx[m + 1]
        Cs[m + 1, m + 1] = 3.0 * _dx[m]
    Cs[K - 3, K - 1] = _dx[-1] * _dx[-1] / dl
    Cs[K - 2, K - 1] = (2.0 * dl + _dx[-1]) * _dx[-2] / dl
    B = S @ Cs
    k = int(np.clip(np.searchsorted(_xk, t, side="right") - 1, 0, K - 2))
    h = _dx[k]
    u = t - _xk[k]
    v_k = B @ invA[k, :]
    v_k1 = B @ invA[k + 1, :]
    w_sk = u**3 / h**2 - 2 * u**2 / h + u
    w_sk1 = u**3 / h**2 - u**2 / h
    w_mk = -2 * u**3 / h**2 + 3 * u**2 / h
    g = w_sk * v_k + w_sk1 * v_k1 + w_mk * S[:, k]
    g[k] += 1.0
    return g


def _window(g: np.ndarray, tau: float = 1e-8) -> tuple:
    """Smallest even-width contiguous [lo, hi) covering {|g| > tau}."""
    idx = np.where(np.abs(g) > tau)[0]
    lo, hi = int(idx.min()), int(idx.max()) + 1
    if (hi - lo) % 2:
        if hi < len(g):
            hi += 1
        else:
            lo -= 1
    return lo, hi


# ---------------------------------------------------------------- program
_PROGRAM_CACHE: dict = {}


def _build_program(C: int, Ci: int, use_cc: bool, variant: int = 0):
    """C = total windowed columns, Ci = columns in the g_i block (rest g_j)."""
    fts = [64, 64, 128, 128, 128] if variant == 8 else FTS
    nt = len(fts)
    foff = [sum(fts[:t]) for t in range(nt)]
    nc = bacc.Bacc(
        "TRN2", target_bir_lowering=False, debug=False, num_devices=NCORES
    )

    xc_ext = nc.declare_dram_parameter("xc", [R, C], f32, isOutput=False)
    if variant == 10:
        x0i_ext = nc.declare_dram_parameter(
            "x0i", [P, FTS[0], Ci], f32, isOutput=False
        )
        x0j_ext = nc.declare_dram_parameter(
            "x0j", [P, FTS[0], C - Ci], f32, isOutput=False
        )
    g2_ext = nc.declare_dram_parameter("g2", [P, C], f32, isOutput=False)
    w1_ext = nc.declare_dram_parameter("w1", [P, 10 * SLAB], f32, isOutput=False)
    soc_ext = nc.declare_dram_parameter("soc", [P, SLAB], f32, isOutput=False)
    b1w2_ext = nc.declare_dram_parameter("b1w2", [1, 20], f32, isOutput=False)
    b2_ext = nc.declare_dram_parameter("b2", [1, 1], f32, isOutput=False)
    # out: [0:R) ndis (natural row order); [R:R+12) partials; [R+12:R+14) finals
    out_ext = nc.declare_dram_parameter("out", [R + 14], f32, isOutput=True)

    if use_cc:
        cc_in = nc.dram_tensor("cc_in", [12], f32)
        cc_out = nc.dram_tensor("cc_out", [12], f32, addr_space="Shared")
        ccw_in = nc.dram_tensor("ccw_in", [4], f32)
        ccw_out = nc.dram_tensor("ccw_out", [4], f32, addr_space="Shared")

    add = mybir.AluOpType.add
    mult = mybir.AluOpType.mult
    X = mybir.AxisListType.X

    with tile.TileContext(nc) as tc:
        with (
            tc.tile_pool(name="const", bufs=1) as cpool,
            tc.tile_pool(name="xc", bufs=2 if variant == 10 else 3) as xpool,
            tc.tile_pool(name="prod", bufs=2) as ppool,
            tc.tile_pool(name="big", bufs=1) as big,
            tc.tile_pool(name="work", bufs=2) as work,
            tc.tile_pool(name="psred", bufs=1, space="PSUM") as psred,
        ):
            ones_t = cpool.tile([P, 1], f32)
            nc.gpsimd.memset(ones_t[:], 1.0)

            if use_cc:
                # warm up the collective path: the first collective pays a
                # global cross-core rendezvous barrier; run it on 16 bytes
                # now so it overlaps the main compute.
                warm = cpool.tile([1, 4], f32)
                nc.gpsimd.memset(warm[:], 0.0)
                nc.sync.dma_start(out=ccw_in[:], in_=warm[:].rearrange("o x -> (o x)"))
                nc.gpsimd.collective_compute(
                    "AllReduce",
                    mybir.AluOpType.add,
                    replica_groups=[list(range(NCORES))],
                    ins=[ccw_in[:].opt()],
                    outs=[ccw_out[:].opt()],
                )

            xsrc = xc_ext[:].rearrange("(p f) c -> p f c", p=P)
            x_tiles = {}

            def load_x(t):
                FT = fts[t]
                fs = slice(foff[t], foff[t] + FT)
                xt = xpool.tile([P, FT, C], f32, tag="x", name=f"x{t}")
                nc.sync.dma_start(out=xt[:], in_=xsrc[:, fs, :])
                return xt

            g2_t = cpool.tile([P, C], f32, name="g2t")
            nc.sync.dma_start(out=g2_t[:], in_=g2_ext[:])
            b1w2_t = cpool.tile([1, 20], f32, name="b1w2t")
            nc.sync.dma_start(out=b1w2_t[:], in_=b1w2_ext[:])
            b2_t = cpool.tile([1, 1], f32, name="b2t")
            nc.sync.dma_start(out=b2_t[:], in_=b2_ext[:])
            if variant == 10:
                x0i_t = xpool.tile([P, fts[0], Ci], f32, name="x0i")
                nc.sync.dma_start(out=x0i_t[:], in_=x0i_ext[:])
                x0j_t = xpool.tile([P, fts[0], C - Ci], f32, name="x0j")
                nc.sync.dma_start(out=x0j_t[:], in_=x0j_ext[:])
                x_tiles[0] = None
                for t in range(1, 3):
                    x_tiles[t] = load_x(t)
            else:
                for t in range(3):
                    x_tiles[t] = load_x(t)

            w1_t = big.tile([P, 10 * SLAB], f32)
            nc.sync.dma_start(out=w1_t[:], in_=w1_ext[:])
            soc_t = big.tile([P, SLAB], f32)
            nc.sync.dma_start(out=soc_t[:], in_=soc_ext[:])

            ri_t = big.tile([P, SLAB], f32)
            rj_t = big.tile([P, SLAB], f32)
            ndis_t = big.tile([P, SLAB], f32)

            NHP = 2 if variant in (5, 6, 7, 8, 9, 10, 11, 13) else nt
            hp = [cpool.tile([P, 12], f32, name=f"hp{t}") for t in range(NHP)]
            for t in range(1, NHP):
                nc.gpsimd.memset(hp[t][:, 10:12], 0.0)

            # soc stats (independent of the spline path)
            nc.vector.tensor_reduce(hp[0][:, 10:11], soc_t[:], X, add)
            s2scr = work.tile([P, SLAB], f32, tag="s2scr")
            nc.gpsimd.tensor_tensor(s2scr[:], soc_t[:], soc_t[:], mult)
            nc.vector.tensor_reduce(hp[0][:, 11:12], s2scr[:], X, add)

            for t in range(nt):
                FT = fts[t]
                fs = slice(foff[t], foff[t] + FT)
                if t in x_tiles:
                    x_t = x_tiles.pop(t)
                else:
                    x_t = load_x(t)
                if x_t is None:
                    x_t = x0i_t  # variant 10, t=0: real sources picked below
                g_bi = g2_t[:, 0:Ci].unsqueeze(1).broadcast_to((P, FT, Ci))
                g_bj = g2_t[:, Ci:C].unsqueeze(1).broadcast_to((P, FT, C - Ci))
                if variant == 5:
                    # GP owns the product pass; DVE owns the reduce chain.
                    # Last tile: DVE takes one window back for balance.
                    if t < nt - 1:
                        prod = ppool.tile([P, FT, C], f32, tag="prod")
                        g_b = g2_t[:].unsqueeze(1).broadcast_to((P, FT, C))
                        nc.gpsimd.tensor_tensor(prod[:], x_t[:], g_b, mult)
                        ri_src = prod[:, :, 0:Ci]
                        rj_src = prod[:, :, Ci:C]
                    else:
                        prodi = ppool.tile([P, FT, Ci], f32, tag="prodi")
                        prodj = ppool.tile([P, FT, C - Ci], f32, tag="prodj")
                        nc.vector.tensor_tensor(prodi[:], x_t[:, :, 0:Ci], g_bi, mult)
                        nc.gpsimd.tensor_tensor(prodj[:], x_t[:, :, Ci:C], g_bj, mult)
                        ri_src = prodi[:]
                        rj_src = prodj[:]
                elif variant == 9:
                    # DVE multiplies only SPL columns; GP takes the rest.
                    SPL = 10
                    prodi = ppool.tile([P, FT, SPL], f32, tag="prodi")
                    prodb = ppool.tile([P, FT, C - SPL], f32, tag="prodb")
                    g_ba = g2_t[:, 0:SPL].unsqueeze(1).broadcast_to((P, FT, SPL))
                    g_bb = g2_t[:, SPL:C].unsqueeze(1).broadcast_to(
                        (P, FT, C - SPL)
                    )
                    nc.vector.tensor_tensor(prodi[:], x_t[:, :, 0:SPL], g_ba, mult)
                    nc.gpsimd.tensor_tensor(prodb[:], x_t[:, :, SPL:C], g_bb, mult)
                    ria = work.tile([P, FT], f32, tag="ria")
                    rib = work.tile([P, FT], f32, tag="rib")
                    nc.vector.tensor_reduce(ria[:], prodi[:], X, add)
                    nc.vector.tensor_reduce(
                        rib[:], prodb[:, :, 0 : Ci - SPL], X, add
                    )
                    nc.vector.tensor_tensor(ri_t[:, fs], ria[:], rib[:], add)
                    nc.vector.tensor_reduce(
                        rj_t[:, fs], prodb[:, :, Ci - SPL : C - SPL], X, add
                    )
                    ri_src = None
                else:
                    prodi = ppool.tile([P, FT, Ci], f32, tag="prodi")
                    prodj = ppool.tile([P, FT, C - Ci], f32, tag="prodj")
                    if variant == 10 and t == 0:
                        xi_src, xj_src = x0i_t[:], x0j_t[:]
                    else:
                        xi_src, xj_src = x_t[:, :, 0:Ci], x_t[:, :, Ci:C]
                    nc.vector.tensor_tensor(prodi[:], xi_src, g_bi, mult)
                    nc.gpsimd.tensor_tensor(prodj[:], xj_src, g_bj, mult)
                    ri_src = prodi[:]
                    rj_src = prodj[:]
                if ri_src is not None:
                    nc.vector.tensor_reduce(ri_t[:, fs], ri_src, X, add)
                    nc.vector.tensor_reduce(rj_t[:, fs], rj_src, X, add)
                num = work.tile([P, FT], f32, tag="num")
                den = work.tile([P, FT], f32, tag="den")
                ndeng = nc.vector if variant in (5, 7) else nc.gpsimd
                ndeng.tensor_tensor(
                    num[:], ri_t[:, fs], rj_t[:, fs], mybir.AluOpType.subtract
                )
                ndeng.tensor_tensor(den[:], ri_t[:, fs], rj_t[:, fs], add)
                if variant == 11:
                    nc.gpsimd.tensor_tensor(
                        ndis_t[:, fs], num[:], den[:], mybir.AluOpType.divide
                    )
                else:
                    rinv = work.tile([P, FT], f32, tag="rinv")
                    nc.vector.reciprocal(rinv[:], den[:])
                    nc.vector.tensor_tensor(ndis_t[:, fs], num[:], rinv[:], mult)
                # W1 @ ndis partials
                if variant in (5, 6, 7, 8, 9, 10, 11, 13):
                    end = foff[t] + FT
                    if end in (SLAB // 2, SLAB):
                        h = 0 if end <= SLAB // 2 else 1
                        hs = slice(h * (SLAB // 2), end)
                        HW = hs.stop - hs.start
                        for m in range(10):
                            scr = work.tile([P, HW], f32, tag="scr2")
                            nc.vector.tensor_tensor(
                                scr[:],
                                w1_t[:, m * SLAB + hs.start : m * SLAB + hs.stop],
                                ndis_t[:, hs],
                                mult,
                            )
                            nc.vector.tensor_reduce(
                                hp[h][:, m : m + 1], scr[:], X, add
                            )
                else:
                    for m in range(10):
                        scr = work.tile([P, FT], f32, tag="scr2")
                        nc.gpsimd.tensor_tensor(
                            scr[:],
                            w1_t[:, m * SLAB + foff[t] : m * SLAB + foff[t] + FT],
                            ndis_t[:, fs],
                            mult,
                        )
                        nc.vector.tensor_reduce(hp[t][:, m : m + 1], scr[:], X, add)

            nc.sync.dma_start(
                out=out_ext[0:R].rearrange("(p f) -> p f", p=P), in_=ndis_t[:]
            )

            # combine per-tile partials, reduce across partitions via TensorE
            acc_hp = hp[0]
            for t in range(1, NHP):
                nxt = cpool.tile([P, 12], f32, name=f"hpacc{t}")
                nc.vector.tensor_tensor(nxt[:], acc_hp[:], hp[t][:], add)
                acc_hp = nxt
            hptot = acc_hp
            pred = psred.tile([12, 1], f32)
            nc.tensor.matmul(pred[:], hptot[:], ones_t[:], start=True, stop=True)
            predsb = cpool.tile([12, 1], f32)
            nc.vector.tensor_copy(predsb[:], pred[:])
            nc.sync.dma_start(
                out=out_ext[R : R + 12].rearrange("(p o) -> p o", o=1),
                in_=predsb[:],
            )

            if use_cc:
                nc.sync.dma_start(
                    out=cc_in[:], in_=predsb[:].rearrange("p o -> (p o)")
                )
                nc.gpsimd.collective_compute(
                    "AllReduce",
                    mybir.AluOpType.add,
                    replica_groups=[list(range(NCORES))],
                    ins=[cc_in[:].opt()],
                    outs=[cc_out[:].opt()],
                )
                redt = cpool.tile([1, 12], f32)
                nc.sync.dma_start(out=redt[:], in_=cc_out[:].unsqueeze(0))
                # tail: h1 = partials + b1 ; leaky ; soc_hat = W2 @ h1 + b2
                h1 = cpool.tile([1, 10], f32)
                nc.vector.tensor_tensor(h1[:], redt[:, 0:10], b1w2_t[:, 0:10], add)
                lk = cpool.tile([1, 10], f32)
                nc.vector.tensor_scalar_mul(lk[:], h1[:], 0.01)
                h1a = cpool.tile([1, 10], f32)
                nc.vector.tensor_tensor(h1a[:], h1[:], lk[:], mybir.AluOpType.max)
                pr = cpool.tile([1, 10], f32)
                nc.vector.tensor_tensor(pr[:], h1a[:], b1w2_t[:, 10:20], mult)
                fin = cpool.tile([1, 2], f32)
                hs = cpool.tile([1, 1], f32)
                nc.vector.tensor_reduce(hs[:], pr[:], X, add)
                nc.vector.tensor_tensor(fin[:, 0:1], hs[:], b2_t[:], add)
                # loss = c*(N*c - 2*S1) + S2
                t1 = cpool.tile([1, 1], f32)
                nc.vector.tensor_scalar_mul(t1[:], fin[:, 0:1], float(NTOT))
                t2 = cpool.tile([1, 1], f32)
                nc.vector.tensor_scalar_mul(t2[:], redt[:, 10:11], 2.0)
                t3 = cpool.tile([1, 1], f32)
                nc.vector.tensor_tensor(
                    t3[:], t1[:], t2[:], mybir.AluOpType.subtract
                )
                t4 = cpool.tile([1, 1], f32)
                nc.vector.tensor_tensor(t4[:], t3[:], fin[:, 0:1], mult)
                nc.vector.tensor_tensor(fin[:, 1:2], t4[:], redt[:, 11:12], add)
                nc.sync.dma_start(
                    out=out_ext[R + 12 : R + 14].unsqueeze(0), in_=fin[:]
                )

    nc.compile()
    return nc


# ---------------------------------------------------------------- entry
def kernel(x, soc, i, j, W1, b1, W2, b2):
    x = np.asarray(x, dtype=np.float32)
    soc = np.ascontiguousarray(np.asarray(soc, dtype=np.float32))
    W1 = np.ascontiguousarray(np.asarray(W1, dtype=np.float32))
    b1 = np.asarray(b1, dtype=np.float32)
    W2 = np.asarray(W2, dtype=np.float32)
    b2 = np.asarray(b2, dtype=np.float32)
    fi, fj = float(np.asarray(i)), float(np.asarray(j))

    variant = int(os.environ.get("BASS_KVARIANT", "6"))
    tau = 1e-7 if variant == 13 else 1e-8
    gi = _build_g(fi)
    gj = _build_g(fj)
    ilo, ihi = _window(gi, tau)
    jlo, jhi = _window(gj, tau)
    Ci, Cj = ihi - ilo, jhi - jlo
    C = Ci + Cj
    giw = gi[ilo:ihi].astype(np.float32)
    gjw = gj[jlo:jhi].astype(np.float32)

    use_cc = bool(int(os.environ.get("BASS_USE_CC", "0")))
    key = (C, Ci, use_cc, variant)
    nc = _PROGRAM_CACHE.get(key)
    if nc is None:
        nc = _build_program(C, Ci, use_cc, variant)
        _PROGRAM_CACHE[key] = nc

    g2_in = np.ascontiguousarray(
        np.broadcast_to(np.concatenate([giw, gjw])[None, :], (P, C))
    )
    b1w2_in = np.concatenate([b1, W2.reshape(-1)]).reshape(1, 20)
    b2_in = b2.reshape(1, 1)

    xc = np.empty((NTOT, C), np.float32)
    xc[:, :Ci] = x[:, ilo:ihi]
    xc[:, Ci:] = x[:, jlo:jhi]

    in_maps = []
    for c in range(NCORES):
        rs = slice(c * R, (c + 1) * R)
        extra = {}
        if variant == 10:
            xv = xc[rs].reshape(P, SLAB, C)
            extra["x0i"] = np.ascontiguousarray(xv[:, : FTS[0], :Ci])
            extra["x0j"] = np.ascontiguousarray(xv[:, : FTS[0], Ci:])
        in_maps.append(
            {
                **extra,
                "xc": xc[rs],
                "g2": g2_in,
                "w1": np.ascontiguousarray(
                    W1[:, rs].reshape(10, P, SLAB).transpose(1, 0, 2)
                ).reshape(P, 10 * SLAB),
                "soc": soc[rs].reshape(P, SLAB),
                "b1w2": b1w2_in,
                "b2": b2_in,
            }
        )

    res = run_bass_kernel_spmd(
        nc,
        in_maps,
        core_ids=list(range(NCORES)),
        trace=bool(int(os.environ.get("BASS_TRACE", "0"))),
    )
    if res.exec_time_ns is not None:
        print(f"HW exec time: {res.exec_time_ns} ns")

    ndis = np.empty(NTOT, dtype=np.float32)
    for c in range(NCORES):
        ndis[c * R : (c + 1) * R] = res.results[c]["out"][:R]

    if use_cc:
        soc_hat = np.array([res.results[0]["out"][R + 12]], dtype=np.float32)
        loss = np.float32(res.results[0]["out"][R + 13])
    else:
        partials = np.sum(
            [res.results[c]["out"][R : R + 12] for c in range(NCORES)], axis=0
        ).astype(np.float32)
        h1 = partials[:10] + b1
        h1 = np.where(h1 > 0, h1, np.float32(0.01) * h1).astype(np.float32)
        c0 = np.float32(W2.reshape(-1) @ h1 + b2[0])
        soc_hat = np.array([c0], dtype=np.float32)
        loss = np.float32(
            np.float32(NTOT) * c0 * c0
            - np.float32(2.0) * c0 * partials[10]
            + partials[11]
        )

    return soc_hat, ndis, np.asarray(loss, dtype=np.float32)
